# revision 51
# baseline (speedup 1.0000x reference)
"""KAN layer (per-edge tiny MLPs) Trainium2 kernel.

Math (per batch b, output o, input i; H=32 hidden):
  h1 = leaky(x[b,i]*W1[o,i,:] + b1[o,i,:])
  z2 = W2[o,i] @ h1 + b2[o,i]           (per-edge [H,H] matmul)
  h2 = leaky(z2)
  edge = W3[o,i]*h2 + b3[o,i]
  out[b,o] = sum_i (bias_w[o,i]*leaky(x[b,i]) + layer_w[o,i]*edge)

Mapping (8 cores, O sharded, 8 output rows per core). Per og block
(og = o*16+g, 4 edges each, tiles are [128=(4j x 32h), B=1024]):
  - h1 in bf16 on DVE as 3 fast ops (z1 = x*w1+b1 and az1 = alpha*z1 via
    tensor_scalar in 4x perf mode ~300ns each, tt max ~510ns); a few og
    on ACT (fused Lrelu with scale/bias, ~1040ns) to balance.
  - MM2 z2 = W2blkdiag^T @ h1 on PE in bf16 (2 x 512-col matmuls).
  - evac h2 = leaky(z2+b2) on ACT (Lrelu, bias fused, ~1040ns).
  - reduce out[o] += v.h2 on PE via st4 [128,8] lhsT (one col per og).
  - MM5 (bias_w*leaky(x) + consts via st5/lxT) emitted mid-stream; the
    first MM4 opens the PSUM accumulation group so PE starts immediately.
  - warmup DMAs spread across SP + gpsimd(SWDGE) queues.
ACT 135.2us, DVE 134.8us, PE 130.1us busy; 144141ns total vs baseline
254731ns (which was ACT/DVE-bound at 187us each).
"""
import sys

sys.path.insert(0, "/opt/trn_rl_repo")

import numpy as np
import ml_dtypes

_B, _I, _O, _H = 1024, 64, 64, 32
_NCORES = 8
_OLOC = _O // _NCORES  # 8 output nodes per core
_ALPHA = 0.01
_NHALF = 512
_BF16 = ml_dtypes.bfloat16

# --- engine assignment knobs ---
def _h1_on_act(og):   # h1 on ACT (fused Lrelu) instead of DVE
    return False

def _h1_abs(og):
    # h1 = alpha*z1 + (1-alpha)*relu(z1) (two fast DVE ts ops, PE adds the
    # two streams via PSUM accumulation - no DVE tensor_tensor max needed).
    # og 0/2 included so the first blocks clear DVE fastest at warmup.
    return og % 3 == 1 or og in (0, 2)

_CACHE = {}


def _build_bass():
    import concourse.bacc as bacc
    import concourse.mybir as mybir
    from concourse.tile import TileContext

    f32 = mybir.dt.float32
    f32r = mybir.dt.float32r
    bf16 = mybir.dt.bfloat16
    AF = mybir.ActivationFunctionType
    ALU = mybir.AluOpType

    nc = bacc.Bacc("TRN2", target_bir_lowering=False, debug=False)

    # wcols = [w1col | b1col | aw1col | ab1col | b2col] (one DMA for warmup)
    wcols_d = nc.declare_dram_parameter("wcols", [128, 5 * 128], f32, isOutput=False)
    xrep_d = nc.declare_dram_parameter("xrep", [2048, _B], bf16, isOutput=False)
    xt65_d = nc.declare_dram_parameter("xt65", [65, _B], f32, isOutput=False)
    w2blk_d = nc.declare_dram_parameter("w2blk", [_OLOC, 128, 16 * 128], bf16,
                                        isOutput=False)
    st4_d = nc.declare_dram_parameter("st4", [128, 128 * _OLOC], bf16, isOutput=False)
    st5_d = nc.declare_dram_parameter("st5", [65, _OLOC], bf16, isOutput=False)
    out_d = nc.declare_dram_parameter("out", [_OLOC, _B], f32, isOutput=True)

    with TileContext(nc) as tc:
        with tc.tile_pool(name="consts", bufs=1) as cpool, \
             tc.tile_pool(name="w2", bufs=2) as w2pool, \
             tc.tile_pool(name="z1", bufs=4) as z1pool, \
             tc.tile_pool(name="az1", bufs=4) as azpool, \
             tc.tile_pool(name="h1", bufs=6) as h1pool, \
             tc.tile_pool(name="h2", bufs=6) as h2pool, \
             tc.tile_pool(name="zps", bufs=3, space="PSUM") as zpool, \
             tc.tile_pool(name="ops", bufs=1, space="PSUM") as opool:

            # --- warmup: spread DMA dispatch across the idle SP and Pool
            # (SWDGE) queues so early compute isn't serialized behind one
            # HWDGE queue. Interleave so xg[g] lands just before og=g needs it.
            xg_t = [cpool.tile([128, _B], bf16, name=f"xg{g}") for g in range(16)]
            wcols_t = cpool.tile([128, 5 * 128], f32)
            st4_t = cpool.tile([128, 128 * _OLOC], bf16)
            xt65_t = cpool.tile([65, _B], f32)
            st5_t = cpool.tile([65, _OLOC], bf16)

            def xgdma(eng, g):
                eng.dma_start(out=xg_t[g][:], in_=xrep_d[g * 128:(g + 1) * 128])

            def w2q(o, q, eng):
                # quarter-slab tiles (4 g's each) so early blocks start sooner
                t = w2pool.tile([128, 4 * 128], bf16, name=f"w2q{q}")
                eng.dma_start(
                    out=t[:], in_=w2blk_d[o, :, q * 512:(q + 1) * 512])
                return t

            w2_t = [[None] * 4]
            xgdma(nc.sync, 0)
            nc.gpsimd.dma_start(out=wcols_t[:], in_=wcols_d[:])
            w2_t[0][0] = w2q(0, 0, nc.sync)
            xgdma(nc.sync, 1)
            w2_t[0][1] = w2q(0, 1, nc.gpsimd)
            nc.gpsimd.dma_start(out=st4_t[:], in_=st4_d[:])
            for g in (2, 4, 6):
                xgdma(nc.sync, g)
            xgdma(nc.gpsimd, 3)
            w2_t[0][2] = w2q(0, 2, nc.gpsimd)
            for g in (7, 8, 10, 12, 14):
                xgdma(nc.sync, g)
            xgdma(nc.gpsimd, 5)
            nc.gpsimd.dma_start(out=xt65_t[:], in_=xt65_d[:])
            nc.gpsimd.dma_start(out=st5_t[:], in_=st5_d[:])
            xgdma(nc.gpsimd, 9)
            w2_t[0][3] = w2q(0, 3, nc.gpsimd)
            for g in (11, 13, 15):
                xgdma(nc.gpsimd, g)

            w1col_t = wcols_t[:, 0 * 128:1 * 128]
            b1col_t = wcols_t[:, 1 * 128:2 * 128]
            aw1col_t = wcols_t[:, 2 * 128:3 * 128]
            ab1col_t = wcols_t[:, 3 * 128:4 * 128]
            b2col_t = wcols_t[:, 4 * 128:5 * 128]

            # lxT = leaky(x) via the relu split on DVE (frees an ACT op)
            azx_t = cpool.tile([65, _B], bf16)
            rx_t = cpool.tile([65, _B], bf16)
            outp = opool.tile([_OLOC, _B], f32)

            first_mm4 = [True]  # first MM4 seeds the PSUM group (start=True)

            def emit_mm4(h2_prev, og_prev, last):
                st = first_mm4[0]
                first_mm4[0] = False
                for half in range(2):
                    sl = slice(half * _NHALF, (half + 1) * _NHALF)
                    nc.tensor.matmul(out=outp[:, sl],
                                     lhsT=st4_t[:, og_prev * 8:(og_prev + 1) * 8],
                                     rhs=h2_prev[:, sl], start=st, stop=last,
                                     skip_group_check=True)

            pending = None  # (h2, og) one block behind, so PE never waits on evac
            for o in range(_OLOC):
                if o + 1 < _OLOC:  # prefetch next W2 slab (double buffered)
                    w2_t.append([w2q(o + 1, q, nc.sync) for q in range(4)])
                for g in range(16):
                    og = o * 16 + g
                    ogc = slice(og, og + 1)
                    if og == 4:
                        # deferred so DVE/PE queues aren't blocked at t=0
                        nc.vector.tensor_scalar(
                            out=azx_t[:], in0=xt65_t[:], scalar1=_ALPHA,
                            scalar2=0.0, op0=ALU.mult, op1=ALU.add)
                        nc.vector.tensor_scalar(
                            out=rx_t[:], in0=azx_t[:],
                            scalar1=(1.0 - _ALPHA) / _ALPHA,
                            scalar2=0.0, op0=ALU.mult, op1=ALU.max)
                    if og == 8:
                        # MM5: bias_w*leaky(x) + consts into outp (mid-group);
                        # leaky(x) = azx + rx summed by PSUM accumulation
                        for half in range(2):
                            sl = slice(half * _NHALF, (half + 1) * _NHALF)
                            nc.tensor.matmul(out=outp[:, sl], lhsT=st5_t[:],
                                             rhs=azx_t[:, sl], start=False,
                                             stop=False, skip_group_check=True)
                            nc.tensor.matmul(out=outp[:, sl], lhsT=st5_t[:],
                                             rhs=rx_t[:, sl], start=False,
                                             stop=False, skip_group_check=True)
                    # --- h1 = leaky(x*w1 + b1), then MM2 z2 = W2blk^T @ h1
                    z2 = zpool.tile([128, _B], f32)
                    lhsT = w2_t[o][g // 4][:, (g % 4) * 128:(g % 4 + 1) * 128]
                    if _h1_abs(og):
                        # h1 = az1 + relu(99*az1), az1 = alpha*z1; PE sums
                        # the two rhs streams in PSUM
                        az1 = z1pool.tile([128, _B], bf16)
                        nc.vector.tensor_scalar(
                            out=az1[:], in0=xg_t[g][:], scalar1=aw1col_t[:, ogc],
                            scalar2=ab1col_t[:, ogc], op0=ALU.mult, op1=ALU.add)
                        r1 = azpool.tile([128, _B], bf16)
                        nc.vector.tensor_scalar(
                            out=r1[:], in0=az1[:], scalar1=(1.0 - _ALPHA) / _ALPHA,
                            scalar2=0.0, op0=ALU.mult, op1=ALU.max)
                        for half in range(2):
                            sl = slice(half * _NHALF, (half + 1) * _NHALF)
                            nc.tensor.matmul(out=z2[:, sl], lhsT=lhsT,
                                             rhs=az1[:, sl], start=True, stop=False)
                            nc.tensor.matmul(out=z2[:, sl], lhsT=lhsT,
                                             rhs=r1[:, sl], start=False, stop=True)
                    else:
                        h1 = h1pool.tile([128, _B], bf16)
                        if _h1_on_act(og):
                            nc.scalar.activation(h1[:], xg_t[g][:], AF.Lrelu,
                                                 bias=b1col_t[:, ogc],
                                                 scale=w1col_t[:, ogc], alpha=_ALPHA)
                        else:
                            z1 = z1pool.tile([128, _B], bf16)
                            nc.vector.tensor_scalar(
                                out=z1[:], in0=xg_t[g][:], scalar1=w1col_t[:, ogc],
                                scalar2=b1col_t[:, ogc], op0=ALU.mult, op1=ALU.add)
                            az1 = azpool.tile([128, _B], bf16)
                            nc.vector.tensor_scalar(
                                out=az1[:], in0=xg_t[g][:], scalar1=aw1col_t[:, ogc],
                                scalar2=ab1col_t[:, ogc], op0=ALU.mult, op1=ALU.add)
                            nc.vector.tensor_tensor(out=h1[:], in0=z1[:],
                                                    in1=az1[:], op=ALU.max)
                        for half in range(2):
                            sl = slice(half * _NHALF, (half + 1) * _NHALF)
                            nc.tensor.matmul(out=z2[:, sl], lhsT=lhsT,
                                             rhs=h1[:, sl], start=True, stop=True)
                    # --- evac h2 = leaky(z2 + b2)
                    h2 = h2pool.tile([128, _B], bf16)
                    nc.scalar.activation(h2[:], z2[:], AF.Lrelu,
                                         bias=b2col_t[:, ogc],
                                         scale=1.0, alpha=_ALPHA)
                    # --- reduce: out[o] += v . h2 (delayed one block)
                    if pending is not None:
                        emit_mm4(*pending, last=False)
                    pending = (h2, og)
            emit_mm4(*pending, last=True)

            # split the drain: DMA of half 0 dispatches (on the idle ACT
            # queue) while DVE still copies half 1; half 1 goes via Pool SWDGE
            outs = cpool.tile([_OLOC, _B], f32)
            nc.vector.tensor_copy(outs[:, 0:_NHALF], outp[:, 0:_NHALF])
            nc.scalar.dma_start(out=out_d[:, 0:_NHALF], in_=outs[:, 0:_NHALF])
            nc.vector.tensor_copy(outs[:, _NHALF:_B], outp[:, _NHALF:_B])
            nc.gpsimd.dma_start(out=out_d[:, _NHALF:_B], in_=outs[:, _NHALF:_B])

    nc.finalize()
    return nc


def _prepare_inputs(x, W1, b1, W2, b2, W3, b3, layer_w, bias_w):
    f = np.float32
    x = np.asarray(x, f)
    xT = np.ascontiguousarray(x.T)                       # [I, B]
    xrep = np.repeat(xT, _H, axis=0).astype(_BF16)       # [2048, B]
    xt65 = np.concatenate([xT, np.ones((1, _B), f)], 0)  # [65, B]

    v = (np.asarray(layer_w, f)[:, :, None] * np.asarray(W3, f))  # [O,I,H]
    w2f = np.asarray(W2, f)

    in_maps = []
    for c in range(_NCORES):
        sl = slice(c * _OLOC, (c + 1) * _OLOC)
        W1c, b1c, b2c = W1[sl], b1[sl], b2[sl]          # [8,64,H]
        W2c = w2f[sl]                                   # [8,64,H,H]
        vc = v[sl]
        lwc, bwc, b3c = layer_w[sl], bias_w[sl], b3[sl]

        # [o, g, j, h] -> partition 32j+h, col o*16+g
        def cols(a):  # a [8, 64, 32] -> [128, 128]
            a = np.asarray(a, f).reshape(_OLOC, 16, 4, _H)
            return np.ascontiguousarray(
                a.transpose(2, 3, 0, 1).reshape(128, 128))

        w1col = cols(W1c)
        b1col = cols(b1c)
        b2col = cols(b2c)
        wcols = np.concatenate(
            [w1col, b1col, _ALPHA * w1col, _ALPHA * b1col, b2col], axis=1)

        # block-diagonal lhsT: blk[og][32j+h, 32j+k] = W2[o,4g+j,k,h]
        W2t = W2c.transpose(0, 1, 3, 2).reshape(_OLOC, 16, 4, _H, _H)
        w2blk = np.zeros((_OLOC, 16, 128, 128), f)
        for j in range(4):
            w2blk[:, :, 32 * j:32 * j + 32, 32 * j:32 * j + 32] = W2t[:, :, j]
        # -> [o, p, g, m] so each (o, p) row is 16*128 contiguous in DRAM
        w2blk = np.ascontiguousarray(
            w2blk.transpose(0, 2, 1, 3).reshape(_OLOC, 128, 16 * 128)
        ).astype(_BF16)

        # st4[og][32j+k, o] = v[o,4g+j,k]
        def stack8b(a):
            a = np.asarray(a, f).reshape(_OLOC, 16, 4 * _H)
            out = np.zeros((128, _OLOC * 16, _OLOC), f)
            for o in range(_OLOC):
                for g in range(16):
                    out[:, o * 16 + g, o] = a[o, g]
            return np.ascontiguousarray(out.reshape(128, 128 * _OLOC))

        st4 = stack8b(vc)

        st5 = np.zeros((65, _OLOC), f)
        st5[:_I, :] = np.asarray(bwc, f).T              # bias_w[o,i] at row i
        const = (np.asarray(lwc, f) * np.asarray(b3c, f)).sum(1)
        st5[_I, :] = const

        in_maps.append({
            "wcols": wcols, "xrep": xrep, "xt65": xt65,
            "w2blk": w2blk, "st4": st4.astype(_BF16),
            "st5": st5.astype(_BF16),
        })
    return in_maps


def kernel(x, W1, b1, W2, b2, W3, b3, layer_w, bias_w):
    from concourse.bass_utils import run_bass_kernel_spmd

    if "nc" not in _CACHE:
        _CACHE["nc"] = _build_bass()
    nc = _CACHE["nc"]

    in_maps = _prepare_inputs(x, W1, b1, W2, b2, W3, b3, layer_w, bias_w)
    res = run_bass_kernel_spmd(nc, in_maps, list(range(_NCORES))).results

    out = np.empty((_B, _O), np.float32)
    for c in range(_NCORES):
        out[:, c * _OLOC:(c + 1) * _OLOC] = res[c]["out"].T
    return out


if __name__ == "__main__":
    # quick self-check against a numpy reference
    rng = np.random.default_rng(0)
    f = np.float32
    inputs = {
        "x": rng.standard_normal((_B, _I)).astype(f),
        "W1": rng.uniform(-1, 1, (_O, _I, _H)).astype(f),
        "b1": rng.uniform(-1, 1, (_O, _I, _H)).astype(f),
        "W2": rng.uniform(-0.2, 0.2, (_O, _I, _H, _H)).astype(f),
        "b2": rng.uniform(-0.2, 0.2, (_O, _I, _H)).astype(f),
        "W3": rng.uniform(-0.2, 0.2, (_O, _I, _H)).astype(f),
        "b3": rng.uniform(-0.2, 0.2, (_O, _I)).astype(f),
        "layer_w": np.ones((_O, _I), f),
        "bias_w": rng.uniform(-0.1, 0.1, (_O, _I)).astype(f),
    }

    def leaky(a):
        return np.where(a >= 0, a, _ALPHA * a)

    def ref(x, W1, b1, W2, b2, W3, b3, layer_w, bias_w):
        h1 = leaky(x[:, None, :, None] * W1 + b1)
        h2 = leaky(np.einsum("boih,oikh->boik", h1, W2) + b2)
        edge = np.einsum("boih,oih->boi", h2, W3) + b3
        edge = bias_w * leaky(x)[:, None, :] + layer_w * edge
        return edge.sum(axis=2)

    expected = ref(**{k: np.asarray(val, np.float64) for k, val in inputs.items()})
    actual = kernel(**inputs)
    err = np.abs(actual - expected).max() / np.abs(expected).max()
    print("rel err:", err)


# revision 53
# speedup vs baseline: 1.0055x; 1.0055x over previous
"""KAN layer (per-edge tiny MLPs) Trainium2 kernel.

Math (per batch b, output o, input i; H=32 hidden):
  h1 = leaky(x[b,i]*W1[o,i,:] + b1[o,i,:])
  z2 = W2[o,i] @ h1 + b2[o,i]           (per-edge [H,H] matmul)
  h2 = leaky(z2)
  edge = W3[o,i]*h2 + b3[o,i]
  out[b,o] = sum_i (bias_w[o,i]*leaky(x[b,i]) + layer_w[o,i]*edge)

Mapping (8 cores, O sharded, 8 output rows per core). Per og block
(og = o*16+g, 4 edges each, tiles are [128=(4j x 32h), B=1024]):
  - h1 in bf16 on DVE as 3 fast ops (z1 = x*w1+b1 and az1 = alpha*z1 via
    tensor_scalar in 4x perf mode ~300ns each, tt max ~510ns); a few og
    on ACT (fused Lrelu with scale/bias, ~1040ns) to balance.
  - MM2 z2 = W2blkdiag^T @ h1 on PE in bf16 (2 x 512-col matmuls).
  - evac h2 = leaky(z2+b2) on ACT (Lrelu, bias fused, ~1040ns).
  - reduce out[o] += v.h2 on PE via st4 [128,8] lhsT (one col per og).
  - MM5 (bias_w*leaky(x) + consts via st5/lxT) emitted mid-stream; the
    first MM4 opens the PSUM accumulation group so PE starts immediately.
  - warmup DMAs spread across SP + gpsimd(SWDGE) queues.
ACT 135.2us, DVE 134.8us, PE 130.1us busy; 144141ns total vs baseline
254731ns (which was ACT/DVE-bound at 187us each).
"""
import sys

sys.path.insert(0, "/opt/trn_rl_repo")

import numpy as np
import ml_dtypes

_B, _I, _O, _H = 1024, 64, 64, 32
_NCORES = 8
_OLOC = _O // _NCORES  # 8 output nodes per core
_ALPHA = 0.01
_NHALF = 512
_BF16 = ml_dtypes.bfloat16

# --- engine assignment knobs ---
def _h1_on_act(og):   # h1 on ACT (fused Lrelu) instead of DVE
    return False

def _h1_abs(og):
    # h1 = alpha*z1 + (1-alpha)*relu(z1) (two fast DVE ts ops, PE adds the
    # two streams via PSUM accumulation - no DVE tensor_tensor max needed).
    # og 0/2 included so the first blocks clear DVE fastest at warmup.
    return og % 3 == 1 or og in (0, 2)

_CACHE = {}


def _build_bass():
    import concourse.bacc as bacc
    import concourse.mybir as mybir
    from concourse.tile import TileContext

    f32 = mybir.dt.float32
    f32r = mybir.dt.float32r
    bf16 = mybir.dt.bfloat16
    AF = mybir.ActivationFunctionType
    ALU = mybir.AluOpType

    nc = bacc.Bacc("TRN2", target_bir_lowering=False, debug=False)

    # wcols = [w1col | b1col | aw1col | ab1col | b2col] (one DMA for warmup)
    wcols_d = nc.declare_dram_parameter("wcols", [128, 5 * 128], f32, isOutput=False)
    xrep_d = nc.declare_dram_parameter("xrep", [2048, _B], bf16, isOutput=False)
    xt65_d = nc.declare_dram_parameter("xt65", [65, _B], f32, isOutput=False)
    w2blk_d = nc.declare_dram_parameter("w2blk", [_OLOC, 128, 16 * 128], bf16,
                                        isOutput=False)
    st4_d = nc.declare_dram_parameter("st4", [128, 128 * _OLOC], bf16, isOutput=False)
    st5_d = nc.declare_dram_parameter("st5", [65, _OLOC], f32r, isOutput=False)
    out_d = nc.declare_dram_parameter("out", [_OLOC, _B], f32, isOutput=True)

    with TileContext(nc) as tc:
        with tc.tile_pool(name="consts", bufs=1) as cpool, \
             tc.tile_pool(name="w2", bufs=2) as w2pool, \
             tc.tile_pool(name="z1", bufs=4) as z1pool, \
             tc.tile_pool(name="az1", bufs=4) as azpool, \
             tc.tile_pool(name="h1", bufs=6) as h1pool, \
             tc.tile_pool(name="h2", bufs=6) as h2pool, \
             tc.tile_pool(name="zps", bufs=3, space="PSUM") as zpool, \
             tc.tile_pool(name="ops", bufs=1, space="PSUM") as opool:

            # --- warmup: spread DMA dispatch across the idle SP and Pool
            # (SWDGE) queues so early compute isn't serialized behind one
            # HWDGE queue. Interleave so xg[g] lands just before og=g needs it.
            xg_t = [cpool.tile([128, _B], bf16, name=f"xg{g}") for g in range(16)]
            wcols_t = cpool.tile([128, 5 * 128], f32)
            st4_t = cpool.tile([128, 128 * _OLOC], bf16)
            xt65_t = cpool.tile([65, _B], f32)
            st5_t = cpool.tile([65, _OLOC], f32r)

            def xgdma(eng, g):
                eng.dma_start(out=xg_t[g][:], in_=xrep_d[g * 128:(g + 1) * 128])

            def w2q(o, q, eng):
                # quarter-slab tiles (4 g's each) so early blocks start sooner
                t = w2pool.tile([128, 4 * 128], bf16, name=f"w2q{q}")
                eng.dma_start(
                    out=t[:], in_=w2blk_d[o, :, q * 512:(q + 1) * 512])
                return t

            w2_t = [[None] * 4]
            xgdma(nc.sync, 0)
            nc.gpsimd.dma_start(out=wcols_t[:], in_=wcols_d[:])
            w2_t[0][0] = w2q(0, 0, nc.sync)
            xgdma(nc.sync, 1)
            w2_t[0][1] = w2q(0, 1, nc.gpsimd)
            nc.gpsimd.dma_start(out=st4_t[:], in_=st4_d[:])
            for g in (2, 4, 6):
                xgdma(nc.sync, g)
            xgdma(nc.gpsimd, 3)
            w2_t[0][2] = w2q(0, 2, nc.gpsimd)
            for g in (7, 8, 10, 12, 14):
                xgdma(nc.sync, g)
            xgdma(nc.gpsimd, 5)
            nc.gpsimd.dma_start(out=xt65_t[:], in_=xt65_d[:])
            nc.gpsimd.dma_start(out=st5_t[:], in_=st5_d[:])
            xgdma(nc.gpsimd, 9)
            w2_t[0][3] = w2q(0, 3, nc.gpsimd)
            for g in (11, 13, 15):
                xgdma(nc.gpsimd, g)

            w1col_t = wcols_t[:, 0 * 128:1 * 128]
            b1col_t = wcols_t[:, 1 * 128:2 * 128]
            aw1col_t = wcols_t[:, 2 * 128:3 * 128]
            ab1col_t = wcols_t[:, 3 * 128:4 * 128]
            b2col_t = wcols_t[:, 4 * 128:5 * 128]

            lxT_t = cpool.tile([65, _B], f32r)
            outp = opool.tile([_OLOC, _B], f32)

            first_mm4 = [True]  # first MM4 seeds the PSUM group (start=True)

            def emit_mm4(h2_prev, og_prev, last):
                st = first_mm4[0]
                first_mm4[0] = False
                for half in range(2):
                    sl = slice(half * _NHALF, (half + 1) * _NHALF)
                    nc.tensor.matmul(out=outp[:, sl],
                                     lhsT=st4_t[:, og_prev * 8:(og_prev + 1) * 8],
                                     rhs=h2_prev[:, sl], start=st, stop=last,
                                     skip_group_check=True)

            pending = None  # (h2, og) one block behind, so PE never waits on evac
            for o in range(_OLOC):
                if o + 1 < _OLOC:  # prefetch next W2 slab (double buffered)
                    w2_t.append([w2q(o + 1, q, nc.sync) for q in range(4)])
                for g in range(16):
                    og = o * 16 + g
                    ogc = slice(og, og + 1)
                    if og == 4:
                        # deferred so ACT/PE queues aren't blocked at t=0
                        nc.scalar.activation(lxT_t[:], xt65_t[:], AF.Lrelu,
                                             bias=0.0, scale=1.0, alpha=_ALPHA)
                    if og == 8:
                        # MM5: bias_w*leaky(x) + consts into outp (mid-group)
                        for half in range(2):
                            sl = slice(half * _NHALF, (half + 1) * _NHALF)
                            nc.tensor.matmul(out=outp[:, sl], lhsT=st5_t[:],
                                             rhs=lxT_t[:, sl], start=False,
                                             stop=False, skip_group_check=True)
                    # --- h1 = leaky(x*w1 + b1), then MM2 z2 = W2blk^T @ h1
                    z2 = zpool.tile([128, _B], f32)
                    lhsT = w2_t[o][g // 4][:, (g % 4) * 128:(g % 4 + 1) * 128]
                    if _h1_abs(og):
                        # h1 = az1 + relu(99*az1), az1 = alpha*z1; PE sums
                        # the two rhs streams in PSUM
                        az1 = z1pool.tile([128, _B], bf16)
                        nc.vector.tensor_scalar(
                            out=az1[:], in0=xg_t[g][:], scalar1=aw1col_t[:, ogc],
                            scalar2=ab1col_t[:, ogc], op0=ALU.mult, op1=ALU.add)
                        r1 = azpool.tile([128, _B], bf16)
                        nc.vector.tensor_scalar(
                            out=r1[:], in0=az1[:], scalar1=(1.0 - _ALPHA) / _ALPHA,
                            scalar2=0.0, op0=ALU.mult, op1=ALU.max)
                        for half in range(2):
                            sl = slice(half * _NHALF, (half + 1) * _NHALF)
                            nc.tensor.matmul(out=z2[:, sl], lhsT=lhsT,
                                             rhs=az1[:, sl], start=True, stop=False)
                            nc.tensor.matmul(out=z2[:, sl], lhsT=lhsT,
                                             rhs=r1[:, sl], start=False, stop=True)
                    else:
                        h1 = h1pool.tile([128, _B], bf16)
                        if _h1_on_act(og):
                            nc.scalar.activation(h1[:], xg_t[g][:], AF.Lrelu,
                                                 bias=b1col_t[:, ogc],
                                                 scale=w1col_t[:, ogc], alpha=_ALPHA)
                        else:
                            z1 = z1pool.tile([128, _B], bf16)
                            nc.vector.tensor_scalar(
                                out=z1[:], in0=xg_t[g][:], scalar1=w1col_t[:, ogc],
                                scalar2=b1col_t[:, ogc], op0=ALU.mult, op1=ALU.add)
                            az1 = azpool.tile([128, _B], bf16)
                            nc.vector.tensor_scalar(
                                out=az1[:], in0=xg_t[g][:], scalar1=aw1col_t[:, ogc],
                                scalar2=ab1col_t[:, ogc], op0=ALU.mult, op1=ALU.add)
                            nc.vector.tensor_tensor(out=h1[:], in0=z1[:],
                                                    in1=az1[:], op=ALU.max)
                        for half in range(2):
                            sl = slice(half * _NHALF, (half + 1) * _NHALF)
                            nc.tensor.matmul(out=z2[:, sl], lhsT=lhsT,
                                             rhs=h1[:, sl], start=True, stop=True)
                    # --- evac h2 = leaky(z2 + b2)
                    h2 = h2pool.tile([128, _B], bf16)
                    if og >= 126:
                        # split the last evacs so the tail MM4/copy/DMA
                        # halves start as soon as their half is ready
                        for half in range(2):
                            sl = slice(half * _NHALF, (half + 1) * _NHALF)
                            nc.scalar.activation(h2[:, sl], z2[:, sl],
                                                 AF.Lrelu, bias=b2col_t[:, ogc],
                                                 scale=1.0, alpha=_ALPHA)
                    else:
                        nc.scalar.activation(h2[:], z2[:], AF.Lrelu,
                                             bias=b2col_t[:, ogc],
                                             scale=1.0, alpha=_ALPHA)
                    # --- reduce: out[o] += v . h2 (delayed one block)
                    if pending is not None:
                        emit_mm4(*pending, last=False)
                    pending = (h2, og)
            emit_mm4(*pending, last=True)

            # split the drain: DMA of half 0 dispatches (on the idle ACT
            # queue) while DVE still copies half 1; half 1 goes via Pool SWDGE
            outs = cpool.tile([_OLOC, _B], f32)
            nc.vector.tensor_copy(outs[:, 0:_NHALF], outp[:, 0:_NHALF])
            nc.scalar.dma_start(out=out_d[:, 0:_NHALF], in_=outs[:, 0:_NHALF])
            nc.vector.tensor_copy(outs[:, _NHALF:_B], outp[:, _NHALF:_B])
            nc.gpsimd.dma_start(out=out_d[:, _NHALF:_B], in_=outs[:, _NHALF:_B])

    nc.finalize()
    return nc


def _prepare_inputs(x, W1, b1, W2, b2, W3, b3, layer_w, bias_w):
    f = np.float32
    x = np.asarray(x, f)
    xT = np.ascontiguousarray(x.T)                       # [I, B]
    xrep = np.repeat(xT, _H, axis=0).astype(_BF16)       # [2048, B]
    xt65 = np.concatenate([xT, np.ones((1, _B), f)], 0)  # [65, B]

    v = (np.asarray(layer_w, f)[:, :, None] * np.asarray(W3, f))  # [O,I,H]
    w2f = np.asarray(W2, f)

    in_maps = []
    for c in range(_NCORES):
        sl = slice(c * _OLOC, (c + 1) * _OLOC)
        W1c, b1c, b2c = W1[sl], b1[sl], b2[sl]          # [8,64,H]
        W2c = w2f[sl]                                   # [8,64,H,H]
        vc = v[sl]
        lwc, bwc, b3c = layer_w[sl], bias_w[sl], b3[sl]

        # [o, g, j, h] -> partition 32j+h, col o*16+g
        def cols(a):  # a [8, 64, 32] -> [128, 128]
            a = np.asarray(a, f).reshape(_OLOC, 16, 4, _H)
            return np.ascontiguousarray(
                a.transpose(2, 3, 0, 1).reshape(128, 128))

        w1col = cols(W1c)
        b1col = cols(b1c)
        b2col = cols(b2c)
        wcols = np.concatenate(
            [w1col, b1col, _ALPHA * w1col, _ALPHA * b1col, b2col], axis=1)

        # block-diagonal lhsT: blk[og][32j+h, 32j+k] = W2[o,4g+j,k,h]
        W2t = W2c.transpose(0, 1, 3, 2).reshape(_OLOC, 16, 4, _H, _H)
        w2blk = np.zeros((_OLOC, 16, 128, 128), f)
        for j in range(4):
            w2blk[:, :, 32 * j:32 * j + 32, 32 * j:32 * j + 32] = W2t[:, :, j]
        # -> [o, p, g, m] so each (o, p) row is 16*128 contiguous in DRAM
        w2blk = np.ascontiguousarray(
            w2blk.transpose(0, 2, 1, 3).reshape(_OLOC, 128, 16 * 128)
        ).astype(_BF16)

        # st4[og][32j+k, o] = v[o,4g+j,k]
        def stack8b(a):
            a = np.asarray(a, f).reshape(_OLOC, 16, 4 * _H)
            out = np.zeros((128, _OLOC * 16, _OLOC), f)
            for o in range(_OLOC):
                for g in range(16):
                    out[:, o * 16 + g, o] = a[o, g]
            return np.ascontiguousarray(out.reshape(128, 128 * _OLOC))

        st4 = stack8b(vc)

        st5 = np.zeros((65, _OLOC), f)
        st5[:_I, :] = np.asarray(bwc, f).T              # bias_w[o,i] at row i
        const = (np.asarray(lwc, f) * np.asarray(b3c, f)).sum(1)
        st5[_I, :] = const

        in_maps.append({
            "wcols": wcols, "xrep": xrep, "xt65": xt65,
            "w2blk": w2blk, "st4": st4.astype(_BF16), "st5": st5,
        })
    return in_maps


def kernel(x, W1, b1, W2, b2, W3, b3, layer_w, bias_w):
    from concourse.bass_utils import run_bass_kernel_spmd

    if "nc" not in _CACHE:
        _CACHE["nc"] = _build_bass()
    nc = _CACHE["nc"]

    in_maps = _prepare_inputs(x, W1, b1, W2, b2, W3, b3, layer_w, bias_w)
    res = run_bass_kernel_spmd(nc, in_maps, list(range(_NCORES))).results

    out = np.empty((_B, _O), np.float32)
    for c in range(_NCORES):
        out[:, c * _OLOC:(c + 1) * _OLOC] = res[c]["out"].T
    return out


if __name__ == "__main__":
    # quick self-check against a numpy reference
    rng = np.random.default_rng(0)
    f = np.float32
    inputs = {
        "x": rng.standard_normal((_B, _I)).astype(f),
        "W1": rng.uniform(-1, 1, (_O, _I, _H)).astype(f),
        "b1": rng.uniform(-1, 1, (_O, _I, _H)).astype(f),
        "W2": rng.uniform(-0.2, 0.2, (_O, _I, _H, _H)).astype(f),
        "b2": rng.uniform(-0.2, 0.2, (_O, _I, _H)).astype(f),
        "W3": rng.uniform(-0.2, 0.2, (_O, _I, _H)).astype(f),
        "b3": rng.uniform(-0.2, 0.2, (_O, _I)).astype(f),
        "layer_w": np.ones((_O, _I), f),
        "bias_w": rng.uniform(-0.1, 0.1, (_O, _I)).astype(f),
    }

    def leaky(a):
        return np.where(a >= 0, a, _ALPHA * a)

    def ref(x, W1, b1, W2, b2, W3, b3, layer_w, bias_w):
        h1 = leaky(x[:, None, :, None] * W1 + b1)
        h2 = leaky(np.einsum("boih,oikh->boik", h1, W2) + b2)
        edge = np.einsum("boih,oih->boi", h2, W3) + b3
        edge = bias_w * leaky(x)[:, None, :] + layer_w * edge
        return edge.sum(axis=2)

    expected = ref(**{k: np.asarray(val, np.float64) for k, val in inputs.items()})
    actual = kernel(**inputs)
    err = np.abs(actual - expected).max() / np.abs(expected).max()
    print("rel err:", err)


# revision 54
# speedup vs baseline: 1.0066x; 1.0011x over previous
"""KAN layer (per-edge tiny MLPs) Trainium2 kernel.

Math (per batch b, output o, input i; H=32 hidden):
  h1 = leaky(x[b,i]*W1[o,i,:] + b1[o,i,:])
  z2 = W2[o,i] @ h1 + b2[o,i]           (per-edge [H,H] matmul)
  h2 = leaky(z2)
  edge = W3[o,i]*h2 + b3[o,i]
  out[b,o] = sum_i (bias_w[o,i]*leaky(x[b,i]) + layer_w[o,i]*edge)

Mapping (8 cores, O sharded, 8 output rows per core). Per og block
(og = o*16+g, 4 edges each, tiles are [128=(4j x 32h), B=1024]):
  - h1 in bf16 on DVE as 3 fast ops (z1 = x*w1+b1 and az1 = alpha*z1 via
    tensor_scalar in 4x perf mode ~300ns each, tt max ~510ns); a few og
    on ACT (fused Lrelu with scale/bias, ~1040ns) to balance.
  - MM2 z2 = W2blkdiag^T @ h1 on PE in bf16 (2 x 512-col matmuls).
  - evac h2 = leaky(z2+b2) on ACT (Lrelu, bias fused, ~1040ns).
  - reduce out[o] += v.h2 on PE via st4 [128,8] lhsT (one col per og).
  - MM5 (bias_w*leaky(x) + consts via st5/lxT) emitted mid-stream; the
    first MM4 opens the PSUM accumulation group so PE starts immediately.
  - warmup DMAs spread across SP + gpsimd(SWDGE) queues.
ACT 135.2us, DVE 134.8us, PE 130.1us busy; 144141ns total vs baseline
254731ns (which was ACT/DVE-bound at 187us each).
"""
import sys

sys.path.insert(0, "/opt/trn_rl_repo")

import numpy as np
import ml_dtypes

_B, _I, _O, _H = 1024, 64, 64, 32
_NCORES = 8
_OLOC = _O // _NCORES  # 8 output nodes per core
_ALPHA = 0.01
_NHALF = 512
_BF16 = ml_dtypes.bfloat16

# --- engine assignment knobs ---
def _h1_on_act(og):   # h1 on ACT (fused Lrelu) instead of DVE
    return False

def _h1_abs(og):
    # h1 = alpha*z1 + (1-alpha)*relu(z1) (two fast DVE ts ops, PE adds the
    # two streams via PSUM accumulation - no DVE tensor_tensor max needed).
    # og 0/2 included so the first blocks clear DVE fastest at warmup.
    return og % 3 == 1 or og in (0, 2)

_CACHE = {}


def _build_bass():
    import concourse.bacc as bacc
    import concourse.mybir as mybir
    from concourse.tile import TileContext

    f32 = mybir.dt.float32
    f32r = mybir.dt.float32r
    bf16 = mybir.dt.bfloat16
    AF = mybir.ActivationFunctionType
    ALU = mybir.AluOpType

    nc = bacc.Bacc("TRN2", target_bir_lowering=False, debug=False)

    # wcols = [w1col | b1col | aw1col | ab1col | b2col] (one DMA for warmup)
    wcols_d = nc.declare_dram_parameter("wcols", [128, 5 * 128], f32, isOutput=False)
    xrep_d = nc.declare_dram_parameter("xrep", [2048, _B], bf16, isOutput=False)
    xt65_d = nc.declare_dram_parameter("xt65", [65, _B], f32, isOutput=False)
    w2blk_d = nc.declare_dram_parameter("w2blk", [_OLOC, 128, 16 * 128], bf16,
                                        isOutput=False)
    st4_d = nc.declare_dram_parameter("st4", [128, 128 * _OLOC], bf16, isOutput=False)
    st5_d = nc.declare_dram_parameter("st5", [65, _OLOC], f32r, isOutput=False)
    out_d = nc.declare_dram_parameter("out", [_OLOC, _B], f32, isOutput=True)

    with TileContext(nc) as tc:
        with tc.tile_pool(name="consts", bufs=1) as cpool, \
             tc.tile_pool(name="w2", bufs=2) as w2pool, \
             tc.tile_pool(name="z1", bufs=4) as z1pool, \
             tc.tile_pool(name="az1", bufs=4) as azpool, \
             tc.tile_pool(name="h1", bufs=6) as h1pool, \
             tc.tile_pool(name="h2", bufs=6) as h2pool, \
             tc.tile_pool(name="zps", bufs=3, space="PSUM") as zpool, \
             tc.tile_pool(name="ops", bufs=1, space="PSUM") as opool:

            # --- warmup: spread DMA dispatch across the idle SP and Pool
            # (SWDGE) queues so early compute isn't serialized behind one
            # HWDGE queue. Interleave so xg[g] lands just before og=g needs it.
            xg_t = [cpool.tile([128, _B], bf16, name=f"xg{g}") for g in range(16)]
            wcols_t = cpool.tile([128, 5 * 128], f32)
            st4_t = cpool.tile([128, 128 * _OLOC], bf16)
            xt65_t = cpool.tile([65, _B], f32)
            st5_t = cpool.tile([65, _OLOC], f32r)

            def xgdma(eng, g):
                eng.dma_start(out=xg_t[g][:], in_=xrep_d[g * 128:(g + 1) * 128])

            def w2q(o, q, eng):
                # quarter-slab tiles (4 g's each) so early blocks start sooner
                t = w2pool.tile([128, 4 * 128], bf16, name=f"w2q{q}")
                eng.dma_start(
                    out=t[:], in_=w2blk_d[o, :, q * 512:(q + 1) * 512])
                return t

            w2_t = [[None] * 4]
            xgdma(nc.sync, 0)
            nc.gpsimd.dma_start(out=wcols_t[:], in_=wcols_d[:])
            w2_t[0][0] = w2q(0, 0, nc.sync)
            xgdma(nc.sync, 1)
            w2_t[0][1] = w2q(0, 1, nc.gpsimd)
            nc.gpsimd.dma_start(out=st4_t[:], in_=st4_d[:])
            for g in (2, 4, 6):
                xgdma(nc.sync, g)
            xgdma(nc.gpsimd, 3)
            w2_t[0][2] = w2q(0, 2, nc.gpsimd)
            for g in (7, 8, 10, 12, 14):
                xgdma(nc.sync, g)
            xgdma(nc.gpsimd, 5)
            nc.gpsimd.dma_start(out=xt65_t[:], in_=xt65_d[:])
            nc.gpsimd.dma_start(out=st5_t[:], in_=st5_d[:])
            xgdma(nc.gpsimd, 9)
            w2_t[0][3] = w2q(0, 3, nc.gpsimd)
            for g in (11, 13, 15):
                xgdma(nc.gpsimd, g)

            w1col_t = wcols_t[:, 0 * 128:1 * 128]
            b1col_t = wcols_t[:, 1 * 128:2 * 128]
            aw1col_t = wcols_t[:, 2 * 128:3 * 128]
            ab1col_t = wcols_t[:, 3 * 128:4 * 128]
            b2col_t = wcols_t[:, 4 * 128:5 * 128]

            lxT_t = cpool.tile([65, _B], f32r)
            outp = opool.tile([_OLOC, _B], f32)

            first_mm4 = [True]  # first MM4 seeds the PSUM group (start=True)

            def emit_mm4(h2_prev, og_prev, last):
                st = first_mm4[0]
                first_mm4[0] = False
                for half in range(2):
                    sl = slice(half * _NHALF, (half + 1) * _NHALF)
                    nc.tensor.matmul(out=outp[:, sl],
                                     lhsT=st4_t[:, og_prev * 8:(og_prev + 1) * 8],
                                     rhs=h2_prev[:, sl], start=st, stop=last,
                                     skip_group_check=True)

            pending = None  # (h2, og) one block behind, so PE never waits on evac
            for o in range(_OLOC):
                if o + 1 < _OLOC:  # prefetch next W2 slab (double buffered)
                    w2_t.append([w2q(o + 1, q, nc.sync) for q in range(4)])
                for g in range(16):
                    og = o * 16 + g
                    ogc = slice(og, og + 1)
                    if og == 4:
                        # deferred so ACT/PE queues aren't blocked at t=0
                        nc.scalar.activation(lxT_t[:], xt65_t[:], AF.Lrelu,
                                             bias=0.0, scale=1.0, alpha=_ALPHA)
                    if og == 8:
                        # MM5: bias_w*leaky(x) + consts into outp (mid-group)
                        for half in range(2):
                            sl = slice(half * _NHALF, (half + 1) * _NHALF)
                            nc.tensor.matmul(out=outp[:, sl], lhsT=st5_t[:],
                                             rhs=lxT_t[:, sl], start=False,
                                             stop=False, skip_group_check=True)
                    # --- h1 = leaky(x*w1 + b1), then MM2 z2 = W2blk^T @ h1
                    z2 = zpool.tile([128, _B], f32)
                    lhsT = w2_t[o][g // 4][:, (g % 4) * 128:(g % 4 + 1) * 128]
                    if _h1_abs(og):
                        # h1 = az1 + relu(99*az1), az1 = alpha*z1; PE sums
                        # the two rhs streams in PSUM
                        az1 = z1pool.tile([128, _B], bf16)
                        nc.vector.tensor_scalar(
                            out=az1[:], in0=xg_t[g][:], scalar1=aw1col_t[:, ogc],
                            scalar2=ab1col_t[:, ogc], op0=ALU.mult, op1=ALU.add)
                        r1 = azpool.tile([128, _B], bf16)
                        nc.vector.tensor_scalar(
                            out=r1[:], in0=az1[:], scalar1=(1.0 - _ALPHA) / _ALPHA,
                            scalar2=0.0, op0=ALU.mult, op1=ALU.max)
                        for half in range(2):
                            sl = slice(half * _NHALF, (half + 1) * _NHALF)
                            nc.tensor.matmul(out=z2[:, sl], lhsT=lhsT,
                                             rhs=az1[:, sl], start=True, stop=False)
                            nc.tensor.matmul(out=z2[:, sl], lhsT=lhsT,
                                             rhs=r1[:, sl], start=False, stop=True)
                    else:
                        h1 = h1pool.tile([128, _B], bf16)
                        if _h1_on_act(og):
                            nc.scalar.activation(h1[:], xg_t[g][:], AF.Lrelu,
                                                 bias=b1col_t[:, ogc],
                                                 scale=w1col_t[:, ogc], alpha=_ALPHA)
                        else:
                            z1 = z1pool.tile([128, _B], bf16)
                            nc.vector.tensor_scalar(
                                out=z1[:], in0=xg_t[g][:], scalar1=w1col_t[:, ogc],
                                scalar2=b1col_t[:, ogc], op0=ALU.mult, op1=ALU.add)
                            az1 = azpool.tile([128, _B], bf16)
                            nc.vector.tensor_scalar(
                                out=az1[:], in0=xg_t[g][:], scalar1=aw1col_t[:, ogc],
                                scalar2=ab1col_t[:, ogc], op0=ALU.mult, op1=ALU.add)
                            nc.vector.tensor_tensor(out=h1[:], in0=z1[:],
                                                    in1=az1[:], op=ALU.max)
                        for half in range(2):
                            sl = slice(half * _NHALF, (half + 1) * _NHALF)
                            nc.tensor.matmul(out=z2[:, sl], lhsT=lhsT,
                                             rhs=h1[:, sl], start=True, stop=True)
                    # --- evac h2 = leaky(z2 + b2)
                    h2 = h2pool.tile([128, _B], bf16)
                    nc.scalar.activation(h2[:], z2[:], AF.Lrelu,
                                         bias=b2col_t[:, ogc],
                                         scale=1.0, alpha=_ALPHA)
                    # --- reduce: out[o] += v . h2 (delayed one block)
                    if pending is not None:
                        emit_mm4(*pending, last=False)
                    pending = (h2, og)
            emit_mm4(*pending, last=True)

            # split the drain: DMA of half 0 dispatches (on the idle ACT
            # queue) while DVE still copies half 1; half 1 goes via Pool SWDGE
            outs = cpool.tile([_OLOC, _B], f32)
            nc.vector.tensor_copy(outs[:, 0:_NHALF], outp[:, 0:_NHALF])
            nc.scalar.dma_start(out=out_d[:, 0:_NHALF], in_=outs[:, 0:_NHALF])
            nc.vector.tensor_copy(outs[:, _NHALF:_B], outp[:, _NHALF:_B])
            nc.gpsimd.dma_start(out=out_d[:, _NHALF:_B], in_=outs[:, _NHALF:_B])

    nc.finalize()
    return nc


def _prepare_inputs(x, W1, b1, W2, b2, W3, b3, layer_w, bias_w):
    f = np.float32
    x = np.asarray(x, f)
    xT = np.ascontiguousarray(x.T)                       # [I, B]
    xrep = np.repeat(xT, _H, axis=0).astype(_BF16)       # [2048, B]
    xt65 = np.concatenate([xT, np.ones((1, _B), f)], 0)  # [65, B]

    v = (np.asarray(layer_w, f)[:, :, None] * np.asarray(W3, f))  # [O,I,H]
    w2f = np.asarray(W2, f)

    in_maps = []
    for c in range(_NCORES):
        sl = slice(c * _OLOC, (c + 1) * _OLOC)
        W1c, b1c, b2c = W1[sl], b1[sl], b2[sl]          # [8,64,H]
        W2c = w2f[sl]                                   # [8,64,H,H]
        vc = v[sl]
        lwc, bwc, b3c = layer_w[sl], bias_w[sl], b3[sl]

        # [o, g, j, h] -> partition 32j+h, col o*16+g
        def cols(a):  # a [8, 64, 32] -> [128, 128]
            a = np.asarray(a, f).reshape(_OLOC, 16, 4, _H)
            return np.ascontiguousarray(
                a.transpose(2, 3, 0, 1).reshape(128, 128))

        w1col = cols(W1c)
        b1col = cols(b1c)
        b2col = cols(b2c)
        wcols = np.concatenate(
            [w1col, b1col, _ALPHA * w1col, _ALPHA * b1col, b2col], axis=1)

        # block-diagonal lhsT: blk[og][32j+h, 32j+k] = W2[o,4g+j,k,h]
        W2t = W2c.transpose(0, 1, 3, 2).reshape(_OLOC, 16, 4, _H, _H)
        w2blk = np.zeros((_OLOC, 16, 128, 128), f)
        for j in range(4):
            w2blk[:, :, 32 * j:32 * j + 32, 32 * j:32 * j + 32] = W2t[:, :, j]
        # -> [o, p, g, m] so each (o, p) row is 16*128 contiguous in DRAM
        w2blk = np.ascontiguousarray(
            w2blk.transpose(0, 2, 1, 3).reshape(_OLOC, 128, 16 * 128)
        ).astype(_BF16)

        # st4[og][32j+k, o] = v[o,4g+j,k]
        def stack8b(a):
            a = np.asarray(a, f).reshape(_OLOC, 16, 4 * _H)
            out = np.zeros((128, _OLOC * 16, _OLOC), f)
            for o in range(_OLOC):
                for g in range(16):
                    out[:, o * 16 + g, o] = a[o, g]
            return np.ascontiguousarray(out.reshape(128, 128 * _OLOC))

        st4 = stack8b(vc)

        st5 = np.zeros((65, _OLOC), f)
        st5[:_I, :] = np.asarray(bwc, f).T              # bias_w[o,i] at row i
        const = (np.asarray(lwc, f) * np.asarray(b3c, f)).sum(1)
        st5[_I, :] = const

        in_maps.append({
            "wcols": wcols, "xrep": xrep, "xt65": xt65,
            "w2blk": w2blk, "st4": st4.astype(_BF16), "st5": st5,
        })
    return in_maps


def kernel(x, W1, b1, W2, b2, W3, b3, layer_w, bias_w):
    from concourse.bass_utils import run_bass_kernel_spmd

    if "nc" not in _CACHE:
        _CACHE["nc"] = _build_bass()
    nc = _CACHE["nc"]

    in_maps = _prepare_inputs(x, W1, b1, W2, b2, W3, b3, layer_w, bias_w)
    res = run_bass_kernel_spmd(nc, in_maps, list(range(_NCORES))).results

    out = np.empty((_B, _O), np.float32)
    for c in range(_NCORES):
        out[:, c * _OLOC:(c + 1) * _OLOC] = res[c]["out"].T
    return out


if __name__ == "__main__":
    # quick self-check against a numpy reference
    rng = np.random.default_rng(0)
    f = np.float32
    inputs = {
        "x": rng.standard_normal((_B, _I)).astype(f),
        "W1": rng.uniform(-1, 1, (_O, _I, _H)).astype(f),
        "b1": rng.uniform(-1, 1, (_O, _I, _H)).astype(f),
        "W2": rng.uniform(-0.2, 0.2, (_O, _I, _H, _H)).astype(f),
        "b2": rng.uniform(-0.2, 0.2, (_O, _I, _H)).astype(f),
        "W3": rng.uniform(-0.2, 0.2, (_O, _I, _H)).astype(f),
        "b3": rng.uniform(-0.2, 0.2, (_O, _I)).astype(f),
        "layer_w": np.ones((_O, _I), f),
        "bias_w": rng.uniform(-0.1, 0.1, (_O, _I)).astype(f),
    }

    def leaky(a):
        return np.where(a >= 0, a, _ALPHA * a)

    def ref(x, W1, b1, W2, b2, W3, b3, layer_w, bias_w):
        h1 = leaky(x[:, None, :, None] * W1 + b1)
        h2 = leaky(np.einsum("boih,oikh->boik", h1, W2) + b2)
        edge = np.einsum("boih,oih->boi", h2, W3) + b3
        edge = bias_w * leaky(x)[:, None, :] + layer_w * edge
        return edge.sum(axis=2)

    expected = ref(**{k: np.asarray(val, np.float64) for k, val in inputs.items()})
    actual = kernel(**inputs)
    err = np.abs(actual - expected).max() / np.abs(expected).max()
    print("rel err:", err)
